# revision 31
# baseline (speedup 1.0000x reference)
"""Trainium2 Bass kernel: dense transformer block (bilinear attention, no softmax).

Reference computation (B=2, S=2048, C=1024, H=16 heads, hd=64, HIDDEN=1024):
    q = split_heads(x @ Wq.T + bq) * hd**-0.5
    k = split_heads(x @ Wk.T + bk)
    v = split_heads(x @ Wv.T + bv)
    out = (q @ k.T) @ v          per (batch, head)   <-- no softmax!
    h = gelu(out @ W1.T + b1);  mlp = h @ W2.T + b2
    y = x + out + mlp

Key algebraic optimization: (q @ k.T) @ v == q @ (k.T @ v). k.T@v is a tiny
[64,64] per head, so attention drops from ~34 GFLOP to ~1 GFLOP.

Sharding (8 cores): rows (batch*seq = 4096) split 512/core; cores 0-3 hold
batch 0, cores 4-7 batch 1. Each core computes q/k/v/MLP for its rows only.
The only cross-core dependency is ktv = k.T@v (contraction over the full 2048
rows of a batch). Each core computes its partial ktv; two 128KB AllGathers
per 4-core batch group (split by head-halves, triggered as soon as each half
of k/v is done) exchange the partials, and the DVE sums the four shards
locally -- AllGather has a much lower ncfw floor than AllReduce and the ~1us
of DVE adds is free. All collective-dependent compute (out', MLP) is ordered
AFTER the whole q projection so the PE has maximal independent work to
overlap the collectives' latency + the PJRT core-start stagger.

Precision: attention path (q/k/v/ktv/out) runs bf16 with fp32 PSUM. The MLP
runs fp8 e4m3 with DoubleRow matmuls (contraction 256/instr, ~1.8x faster):
out and h are evicted as fp8 (scale 1), W1/W2 are pre-scaled x512 on host
(fp8 exponent range) and the 1/512 is folded into the PSUM-eviction scale.
Measured accuracy ~1.3e-2 absmax rel vs the fp32 reference (limit 2e-2).
Output y is written bf16 and cast to f32 on host.
"""

import sys
import types

sys.path.insert(0, "/opt/trn_rl_repo")

import numpy as np
import ml_dtypes

# ---------------------------------------------------------------------------
# NTFF profile hook shim (this image's antenv lacks axon_hooks; inject it so
# run_bass_kernel_spmd(trace=True) can profile). Harmless when unused.
# ---------------------------------------------------------------------------
if "antenv.axon_hooks" not in sys.modules:
    _m = types.ModuleType("antenv.axon_hooks")
    _m._hook = None
    _m.set_axon_ntff_profile_hook = lambda h: setattr(_m, "_hook", h)
    _m.get_axon_ntff_profile_hook = lambda: _m._hook
    sys.modules["antenv.axon_hooks"] = _m
    try:
        import antenv

        antenv.axon_hooks = _m
        from trn_agent_boot.trn_boot import _ntff_profile_via_ctypes

        _m.set_axon_ntff_profile_hook(
            _ntff_profile_via_ctypes("/opt/axon/libaxon_pjrt.so")
        )
    except Exception:
        pass

import concourse.bass as bass
import concourse.mybir as mybir
import concourse.tile as tile
from concourse import bacc
from concourse import bass_utils

bass_utils.upload_artifacts = lambda tmpdir: tmpdir  # no fish bucket here
from concourse.bass_utils import run_bass_kernel_spmd

BF16 = mybir.dt.bfloat16
FP8 = mybir.dt.float8e4
F32 = mybir.dt.float32
AF = mybir.ActivationFunctionType
ALU = mybir.AluOpType
DR = mybir.MatmulPerfMode.DoubleRow

B, S, C = 2, 2048, 1024
NH, HD = 16, 64
SCALE = HD ** -0.5
NCORES = 8
R = (B * S) // NCORES        # 512 rows per core
P = 128
CH = C // P                  # 8 contraction chunks
RCH = R // P                 # 4 row chunks per core
HP = NH // 2                 # 8 head-pairs (one 128-partition chunk each)
W8S = 512.0                  # host pre-scale on fp8 W1/W2

_CACHE = {}


def _build(kv_bias: bool):
    """Build + compile the 8-core SPMD program. Returns the Bacc graph."""
    nc = bacc.Bacc("TRN2", target_bir_lowering=False, debug=False, num_devices=NCORES)

    # ---- DRAM I/O (per-core shapes; data differs per core) ----
    xtb_d = nc.dram_tensor("xtb", [P, CH * R], BF16, kind="ExternalInput")
    wq_d = nc.dram_tensor("wq", [P, CH * C], BF16, kind="ExternalInput")
    wk_d = nc.dram_tensor("wk", [P, CH * C], BF16, kind="ExternalInput")
    wv_d = nc.dram_tensor("wv", [P, CH * C], BF16, kind="ExternalInput")
    w1_d = nc.dram_tensor("w1", [P, CH * C], FP8, kind="ExternalInput")
    w2_d = nc.dram_tensor("w2", [P, CH * C], FP8, kind="ExternalInput")
    bqs_d = nc.dram_tensor("bqs", [P, CH], F32, kind="ExternalInput")
    b1r_d = nc.dram_tensor("b1r", [P, CH], F32, kind="ExternalInput")
    b2r_d = nc.dram_tensor("b2r", [P, CH], F32, kind="ExternalInput")
    if kv_bias:
        bkr_d = nc.dram_tensor("bkr", [1, C], BF16, kind="ExternalInput")
        bvr_d = nc.dram_tensor("bvr", [1, C], BF16, kind="ExternalInput")
    yt_d = nc.dram_tensor("yt", [P, CH * R], BF16, kind="ExternalOutput")

    # Internal DRAM for the two ktv AllGathers (one per head-half; each core
    # contributes its tightly-packed 64KB partial [64, 8 heads x 64], receives
    # the 4-core group's 256KB concat on the partition axis, sums shards
    # locally and expands into the block-diagonal layout).
    KF = (HP // 2) * HD          # 256: packed free size per half
    ktv_loc = [nc.dram_tensor(f"ktv_loc{i}", [P, KF], BF16) for i in (0, 1)]
    ktv_gth = [nc.dram_tensor(f"ktv_gth{i}", [4 * P, KF], BF16) for i in (0, 1)]
    groups = [[0, 1, 2, 3], [4, 5, 6, 7]]

    with tile.TileContext(nc) as tc:
        with (
            tc.tile_pool(name="persist", bufs=1) as pp,
            tc.tile_pool(name="ypool", bufs=3) as yp,
            tc.tile_pool(name="psum", bufs=6, space="PSUM") as psp,
            # W2 gets its own 2-bank pool so its 8 accumulation groups can't
            # all run ahead of the evictions (forces pipelined y evictions
            # instead of a serialized tail).
            tc.tile_pool(name="psum2", bufs=2, space="PSUM") as ps2,
        ):
            # ---- persistent SBUF tiles ----
            # xtb: chunk c at [:, c*R:(c+1)*R]. wk/wv: half-blocked layout,
            # (oh, c) slice at [:, oh*CH*512 + c*512 :+ 512] (matches the
            # host packing; enables large consolidated input DMAs).
            xtb_t = pp.tile([P, CH * R], BF16, name="xtb_t")
            wk_t = pp.tile([P, CH * C], BF16, name="wk_t")
            wv_t = pp.tile([P, CH * C], BF16, name="wv_t")
            xtb = [xtb_t[:, c * R : (c + 1) * R] for c in range(CH)]
            HB = CH * 512                # fh/sh block size in the wk/wv layout
            wq = pp.tile([P, CH * C], BF16, name="wq_sb")
            w1 = pp.tile([P, CH * C], FP8, name="w1_sb")
            w2 = pp.tile([P, CH * C], FP8, name="w2_sb")
            bqs = pp.tile([P, CH], F32, name="bqs_sb")
            b1r = pp.tile([P, CH], F32, name="b1r_sb")
            b2r = pp.tile([P, CH], F32, name="b2r_sb")
            k_sb = [pp.tile([P, C], BF16, name=f"k_sb{i}") for i in range(RCH)]
            v_sb = [pp.tile([P, C], BF16, name=f"v_sb{i}") for i in range(RCH)]
            q_sb = [pp.tile([P, R], BF16, name=f"q_sb{i}") for i in range(HP)]
            # ox[m] = out' + b2 + x' (residual pre-sum, built during out')
            ox = [pp.tile([P, R], BF16, name=f"ox{i}") for i in range(HP)]
            out8 = pp.tile([P, CH * R], FP8, name="out8_sb")
            h8 = pp.tile([P, CH * R], FP8, name="h8_sb")
            ktv_acc = [pp.tile([P, KF], BF16, name=f"ktv_acc{i}") for i in (0, 1)]
            gsb = [pp.tile([P, 4 * KF], BF16, name=f"gsb{i}") for i in (0, 1)]
            gt = [pp.tile([P, KF], BF16, name=f"gt{i}") for i in (0, 1)]
            ktv_bb = pp.tile([P, HP * P], BF16, name="ktv_bb")
            if kv_bias:
                ones = pp.tile([1, P], BF16, name="ones_sb")
                bkr = pp.tile([1, C], BF16, name="bkr_sb")
                bvr = pp.tile([1, C], BF16, name="bvr_sb")

            # ---- input DMAs (dual HWDGE queues, consolidated) ----
            # Small DMAs pay ~1KB/partition-row descriptors (~190 GB/s); big
            # spans get 4-8KB descriptors (~340 GB/s). Two parallel queue
            # rows: sync carries wk/wq/w1/w2 (+readbacks), scalar carries
            # xtb/wv (+ktv bounce + y out). First chunks ship small so
            # compute starts immediately. wk/wv DRAM layout is half-blocked:
            # fh chunks 0-7, then sh chunks 0-7.
            for c in range(4):
                nc.scalar.dma_start(
                    out=xtb_t[:, c * R : (c + 1) * R],
                    in_=xtb_d[:, c * R : (c + 1) * R],
                )
                nc.sync.dma_start(
                    out=wk_t[:, c * 512 : (c + 1) * 512],
                    in_=wk_d[:, c * 512 : (c + 1) * 512],
                )
            nc.scalar.dma_start(
                out=xtb_t[:, 4 * R : CH * R], in_=xtb_d[:, 4 * R : CH * R]
            )
            nc.sync.dma_start(
                out=wk_t[:, 4 * 512 : HB], in_=wk_d[:, 4 * 512 : HB]
            )
            nc.scalar.dma_start(out=wv_t[:, 0:HB], in_=wv_d[:, 0:HB])
            nc.sync.dma_start(out=wk_t[:, HB : 2 * HB], in_=wk_d[:, HB : 2 * HB])
            nc.scalar.dma_start(out=wv_t[:, HB : 2 * HB], in_=wv_d[:, HB : 2 * HB])
            if kv_bias:
                nc.vector.memset(ones[:], 1.0)
                nc.sync.dma_start(out=bkr[:], in_=bkr_d[:])
                nc.sync.dma_start(out=bvr[:], in_=bvr_d[:])
            nc.sync.dma_start(out=wq[:], in_=wq_d[:])
            nc.sync.dma_start(out=bqs[:], in_=bqs_d[:])
            nc.sync.dma_start(out=w1[:], in_=w1_d[:])
            nc.sync.dma_start(out=b1r[:], in_=b1r_d[:])
            nc.sync.dma_start(out=w2[:], in_=w2_d[:])
            nc.sync.dma_start(out=b2r[:], in_=b2r_d[:])
            # zero the ktv block-diagonal tile (only head-diagonal blocks are
            # filled from the gathered sum)
            nc.vector.memset(ktv_bb[:], 0.0)

            # ---- k, v projections (row-major [r, o]) ----
            # contraction-OUTER loops, split by output half (oh): compute
            # k(oh) then v(oh), then the 4 head-pair ktv blocks of that half,
            # and launch that half's AllGather immediately.
            def proj_kv(w_t, brow, dst, oh):
                pss = [
                    psp.tile([P, 512], F32, name="ps", tag="ps")
                    for _ in range(RCH)
                ]
                for c in range(CH):
                    for ri in range(RCH):
                        nc.tensor.matmul(
                            pss[ri][:],
                            xtb_t[:, c * R + ri * P : c * R + (ri + 1) * P],
                            w_t[:, oh * HB + c * 512 : oh * HB + (c + 1) * 512],
                            start=(c == 0),
                            stop=(c == CH - 1 and not kv_bias),
                        )
                for ri in range(RCH):
                    ps = pss[ri]
                    if kv_bias:
                        nc.tensor.matmul(
                            ps[:],
                            ones[:1, :],
                            brow[:1, oh * 512 : (oh + 1) * 512],
                            start=False,
                            stop=True,
                        )
                    dst_ap = dst[ri][:, oh * 512 : (oh + 1) * 512]
                    if ri % 2 == 0:
                        nc.vector.tensor_copy(dst_ap, ps[:])
                    else:
                        nc.scalar.activation(dst_ap, ps[:], AF.Copy)

            for oh in range(2):
                proj_kv(wk_t, bkr if kv_bias else None, k_sb, oh)
                proj_kv(wv_t, bvr if kv_bias else None, v_sb, oh)

                # partial ktv for this half: head-pairs packed [128,128];
                # diagonal blocks are the per-head ktvs, off-diagonal is
                # garbage. Pack the diagonal strips tightly ([128, 4x64]:
                # even heads on partitions 0-63, odd heads on 64-127) and
                # trigger the AllGather.
                with tc.high_priority(offset=400):
                    pk = psp.tile([P, 512], F32, name="ps", tag="ps")
                    for hpl in range(HP // 2):
                        hp = oh * (HP // 2) + hpl
                        for ri in range(RCH):
                            nc.tensor.matmul(
                                pk[:, hpl * P : (hpl + 1) * P],
                                k_sb[ri][:, hp * P : (hp + 1) * P],
                                v_sb[ri][:, hp * P : (hp + 1) * P],
                                start=(ri == 0),
                                stop=(ri == RCH - 1),
                            )
                    pk_v = pk.rearrange("p (hp t d) -> p hp t d", hp=HP // 2, t=2, d=HD)
                    acc_v = ktv_acc[oh].rearrange(
                        "p (hp d) -> p hp d", hp=HP // 2, d=HD
                    )
                    # split the two strip copies across engines (parallel)
                    nc.vector.tensor_copy(
                        acc_v[0:HD, :, :], pk_v[0:HD, :, 0, :]
                    )
                    nc.scalar.activation(
                        acc_v[HD:P, :, :], pk_v[HD:P, :, 1, :], AF.Copy
                    )
                    nc.scalar.dma_start(out=ktv_loc[oh][:], in_=ktv_acc[oh][:])
                    nc.gpsimd.collective_compute(
                        "AllGather",
                        ALU.bypass,
                        replica_groups=groups,
                        ins=[ktv_loc[oh][:]],
                        outs=[ktv_gth[oh][:]],
                    )

            # ---- q' projection (feature-major [o, r]) ----
            # All collective-dependent work is ordered after it.
            for m in range(CH):
                ps = psp.tile([P, 512], F32, name="ps", tag="ps")
                for c in range(CH):
                    nc.tensor.matmul(
                        ps[:],
                        wq[:, c * C + m * P : c * C + (m + 1) * P],
                        xtb_t[:, c * R : (c + 1) * R],
                        start=(c == 0),
                        stop=(c == CH - 1),
                    )
                # q evictions stay on ACT: the DVE FIFO must remain clear for
                # the gather-sum adds that gate out' right after q finishes
                nc.scalar.activation(
                    q_sb[m][:], ps[:], AF.Identity, bias=bqs[:, m : m + 1]
                )

            # ---- AllGather readback, shard sum + block-diag expansion ----
            def gather_sum(oh):
                with tc.high_priority(offset=300):
                    # single strided DMA: DRAM [4*P, KF] -> SBUF [P, 4*KF]
                    nc.scalar.dma_start(
                        out=gsb[oh].rearrange("p (r f) -> p r f", r=4),
                        in_=ktv_gth[oh].rearrange("(r p) f -> p r f", r=4),
                    )
                    g = gsb[oh]
                    nc.vector.tensor_add(
                        gt[oh][:], g[:, 0:KF], g[:, KF : 2 * KF]
                    )
                    nc.vector.tensor_add(
                        g[:, 0:KF], g[:, 2 * KF : 3 * KF], g[:, 3 * KF : 4 * KF]
                    )
                    # final add writes straight into the zeroed block-diagonal
                    # layout: even heads -> partitions 0-63 of t=0 blocks,
                    # odd heads -> partitions 64-127 of t=1 blocks.
                    bb_v = ktv_bb[:, oh * 512 : (oh + 1) * 512].rearrange(
                        "p (hp t d) -> p hp t d", hp=HP // 2, t=2, d=HD
                    )
                    gt_v = gt[oh].rearrange("p (hp d) -> p hp d", hp=HP // 2, d=HD)
                    g0_v = g[:, 0:KF].rearrange("p (hp d) -> p hp d", hp=HP // 2, d=HD)
                    nc.vector.tensor_add(
                        bb_v[0:HD, :, 0, :], gt_v[0:HD, :, :], g0_v[0:HD, :, :]
                    )
                    nc.vector.tensor_add(
                        bb_v[HD:P, :, 1, :], gt_v[HD:P, :, :], g0_v[HD:P, :, :]
                    )

            # ---- out' = blockdiag(ktv).T @ q' + fp8 MLP ----
            HPH = HP // 2

            def out_chunk(hp):
                ps = psp.tile([P, 512], F32, name="ps", tag="ps")
                nc.tensor.matmul(
                    ps[:],
                    ktv_bb[:, hp * P : (hp + 1) * P],
                    q_sb[hp][:],
                    start=True,
                    stop=True,
                )
                # ox = (out' + b2) + x' -- the full non-MLP part of y, so the
                # final y eviction is a single DVE op per chunk. out8 (the
                # fp8 MLP operand) alternates engines: it gates the W1
                # matmuls, so halving its eviction latency matters.
                nc.vector.scalar_tensor_tensor(
                    ox[hp][:], ps[:], b2r[:, hp : hp + 1],
                    xtb_t[:, hp * R : (hp + 1) * R], ALU.add, ALU.add,
                )
                if hp % 2 == 0:
                    nc.scalar.activation(
                        out8[:, hp * R : (hp + 1) * R], ps[:], AF.Copy
                    )
                else:
                    nc.vector.tensor_copy(out8[:, hp * R : (hp + 1) * R], ps[:])

            def w1v(cp, j):
                sl = w1[:, cp * 2 * C : (cp + 1) * 2 * C]
                return sl.rearrange("p (t f) -> p t f", t=2)[
                    :, :, j * P : (j + 1) * P
                ]

            def o8v(cp):
                sl = out8[:, cp * 2 * R : (cp + 1) * 2 * R]
                return sl.rearrange("p (t f) -> p t f", t=2)

            gather_sum(0)
            for hp in range(HPH):
                out_chunk(hp)
            # h' partial: j-groups 0-4 over the available c-pairs 0,1
            # (5 held PSUM groups; the 6-bank pool keeps one rotating)
            NHLD = 5
            hps = []
            for j in range(NHLD):
                ps = psp.tile([P, 512], F32, name="ps", tag="ps")
                hps.append(ps)
                for cp in range(2):
                    nc.tensor.matmul(
                        ps[:], w1v(cp, j), o8v(cp),
                        start=(cp == 0), stop=False, perf_mode=DR,
                    )
            gather_sum(1)
            for hp in range(HPH, HP):
                out_chunk(hp)

            # ---- finish h' = gelu((W1*512) out' /512 + b1) -> fp8 ----
            for j in range(NHLD):
                ps = hps[j]
                for cp in range(2, 4):
                    nc.tensor.matmul(
                        ps[:], w1v(cp, j), o8v(cp),
                        start=False, stop=(cp == 3), perf_mode=DR,
                    )
                nc.scalar.activation(
                    h8[:, j * R : (j + 1) * R], ps[:], AF.Gelu,
                    bias=b1r[:, j : j + 1], scale=1.0 / W8S,
                )
            for j in range(NHLD, CH):
                ps = psp.tile([P, 512], F32, name="ps", tag="ps")
                for cp in range(4):
                    nc.tensor.matmul(
                        ps[:], w1v(cp, j), o8v(cp),
                        start=(cp == 0), stop=(cp == 3), perf_mode=DR,
                    )
                nc.scalar.activation(
                    h8[:, j * R : (j + 1) * R], ps[:], AF.Gelu,
                    bias=b1r[:, j : j + 1], scale=1.0 / W8S,
                )

            # ---- MLP out + residual: y' = (W2 h')/512 + b2 + out' + x' ----
            def w2v(jp, m):
                sl = w2[:, jp * 2 * C : (jp + 1) * 2 * C]
                return sl.rearrange("p (t f) -> p t f", t=2)[
                    :, :, m * P : (m + 1) * P
                ]

            def h8v(jp):
                sl = h8[:, jp * 2 * R : (jp + 1) * 2 * R]
                return sl.rearrange("p (t f) -> p t f", t=2)

            for m in range(CH):
                ps = ps2.tile([P, 512], F32, name="ps2", tag="ps2")
                for jp in range(4):
                    nc.tensor.matmul(
                        ps[:], w2v(jp, m), h8v(jp),
                        start=(jp == 0), stop=(jp == 3), perf_mode=DR,
                    )
                # y = psum/512 + (out' + b2 + x'): one DVE op per chunk. The
                # last chunk's eviction + DMA is the kernel tail -- split it
                # in half so the store starts sooner.
                y_t = yp.tile([P, 512], BF16, name="y_t")
                if m < CH - 1:
                    nc.vector.scalar_tensor_tensor(
                        y_t[:], ps[:], 1.0 / W8S, ox[m][:], ALU.mult, ALU.add
                    )
                    nc.scalar.dma_start(out=yt_d[:, m * R : (m + 1) * R], in_=y_t[:])
                else:
                    for h in range(2):
                        sl = slice(h * 256, (h + 1) * 256)
                        nc.vector.scalar_tensor_tensor(
                            y_t[:, sl], ps[:, sl], 1.0 / W8S, ox[m][:, sl],
                            ALU.mult, ALU.add,
                        )
                        nc.scalar.dma_start(
                            out=yt_d[:, m * R + h * 256 : m * R + (h + 1) * 256],
                            in_=y_t[:, sl],
                        )

    nc.compile()
    return nc


def _get_nc(kv_bias: bool):
    key = ("nc", kv_bias)
    if key not in _CACHE:
        _CACHE[key] = _build(kv_bias)
    return _CACHE[key]


def _pack_pf(a):
    """[CH*P, F] row-major -> [P, CH*F] (partition-chunk packing)."""
    n, f = a.shape
    ch = n // P
    return np.ascontiguousarray(a.reshape(ch, P, f).transpose(1, 0, 2).reshape(P, ch * f))


def _prep_inputs(x, Wq, bq, Wk, bk, Wv, bv, W1, b1, W2, b2, kv_bias):
    bf = ml_dtypes.bfloat16
    f8 = ml_dtypes.float8_e4m3
    def _half_block(a):
        # [P, CH*C] chunk-major -> [P, (2, CH, 512)] half-blocked
        return np.ascontiguousarray(
            a.reshape(P, CH, 2, 512).transpose(0, 2, 1, 3).reshape(P, CH * C)
        )

    wq_p = _pack_pf((Wq.T * SCALE).astype(np.float32)).astype(bf)
    wk_p = _half_block(_pack_pf(np.ascontiguousarray(Wk.T))).astype(bf)
    wv_p = _half_block(_pack_pf(np.ascontiguousarray(Wv.T))).astype(bf)
    w1_p = np.clip(_pack_pf(np.ascontiguousarray(W1.T)) * W8S, -240, 240).astype(f8)
    w2_p = np.clip(_pack_pf(np.ascontiguousarray(W2.T)) * W8S, -240, 240).astype(f8)
    bqs = np.ascontiguousarray((bq * SCALE).astype(np.float32).reshape(CH, P).T)
    b1r = np.ascontiguousarray(b1.astype(np.float32).reshape(CH, P).T)
    b2r = np.ascontiguousarray(b2.astype(np.float32).reshape(CH, P).T)

    xf = x.reshape(B * S, C)
    in_maps = []
    for core in range(NCORES):
        xs = xf[core * R : (core + 1) * R]           # [R, C]
        xt = _pack_pf(np.ascontiguousarray(xs.T))    # [P, CH*R] f32
        m = {
            "xtb": xt.astype(bf),
            "wq": wq_p,
            "wk": wk_p,
            "wv": wv_p,
            "w1": w1_p,
            "w2": w2_p,
            "bqs": bqs,
            "b1r": b1r,
            "b2r": b2r,
        }
        if kv_bias:
            m["bkr"] = bk.astype(bf).reshape(1, C)
            m["bvr"] = bv.astype(bf).reshape(1, C)
        in_maps.append(m)
    return in_maps


def _unpack_out(results):
    y = np.empty((B * S, C), np.float32)
    for core in range(NCORES):
        yt = np.asarray(results[core]["yt"], dtype=np.float32)   # [P, CH*R]
        blk = yt.reshape(P, CH, R).transpose(1, 0, 2).reshape(C, R)
        y[core * R : (core + 1) * R] = blk.T
    return y.reshape(B, S, C)


def _run(inputs, trace=False, trace_cores=None):
    x = np.asarray(inputs["x"], np.float32)
    args = [np.asarray(inputs[k], np.float32) for k in
            ("Wq", "bq", "Wk", "bk", "Wv", "bv", "W1", "b1", "W2", "b2")]
    kv_bias = bool(np.any(args[3]) or np.any(args[5]))
    nc = _get_nc(kv_bias)
    in_maps = _prep_inputs(x, *args, kv_bias)
    res = run_bass_kernel_spmd(
        nc, in_maps, core_ids=list(range(NCORES)), trace=trace,
        trace_cores=trace_cores,
    )
    return _unpack_out(res.results), res


def kernel(**inputs) -> np.ndarray:
    out, _ = _run(inputs, trace=False)
    return out


def kernel_profiled(**inputs):
    """Returns (output, exec_time_ns) using neuron-profile NTFF timing."""
    out, res = _run(inputs, trace=True)
    return out, res.exec_time_ns


# revision 32
# speedup vs baseline: 1.0321x; 1.0321x over previous
"""Trainium2 Bass kernel: dense transformer block (bilinear attention, no softmax).

Reference computation (B=2, S=2048, C=1024, H=16 heads, hd=64, HIDDEN=1024):
    q = split_heads(x @ Wq.T + bq) * hd**-0.5
    k = split_heads(x @ Wk.T + bk)
    v = split_heads(x @ Wv.T + bv)
    out = (q @ k.T) @ v          per (batch, head)   <-- no softmax!
    h = gelu(out @ W1.T + b1);  mlp = h @ W2.T + b2
    y = x + out + mlp

Key algebraic optimization: (q @ k.T) @ v == q @ (k.T @ v). k.T@v is a tiny
[64,64] per head, so attention drops from ~34 GFLOP to ~1 GFLOP.

Sharding (8 cores): rows (batch*seq = 4096) split 512/core; cores 0-3 hold
batch 0, cores 4-7 batch 1. Each core computes q/k/v/MLP for its rows only.
The only cross-core dependency is ktv = k.T@v (contraction over the full 2048
rows of a batch). Each core computes its partial ktv; two 128KB AllGathers
per 4-core batch group (split by head-halves, triggered as soon as each half
of k/v is done) exchange the partials, and the DVE sums the four shards
locally -- AllGather has a much lower ncfw floor than AllReduce and the ~1us
of DVE adds is free. All collective-dependent compute (out', MLP) is ordered
AFTER the whole q projection so the PE has maximal independent work to
overlap the collectives' latency + the PJRT core-start stagger.

Precision: attention path (q/k/v/ktv/out) runs bf16 with fp32 PSUM. The MLP
runs fp8 e4m3 with DoubleRow matmuls (contraction 256/instr, ~1.8x faster):
out and h are evicted as fp8 (scale 1), W1/W2 are pre-scaled x512 on host
(fp8 exponent range) and the 1/512 is folded into the PSUM-eviction scale.
Measured accuracy ~1.3e-2 absmax rel vs the fp32 reference (limit 2e-2).
Output y is written bf16 and cast to f32 on host.
"""

import sys
import types

sys.path.insert(0, "/opt/trn_rl_repo")

import numpy as np
import ml_dtypes

# ---------------------------------------------------------------------------
# NTFF profile hook shim (this image's antenv lacks axon_hooks; inject it so
# run_bass_kernel_spmd(trace=True) can profile). Harmless when unused.
# ---------------------------------------------------------------------------
if "antenv.axon_hooks" not in sys.modules:
    _m = types.ModuleType("antenv.axon_hooks")
    _m._hook = None
    _m.set_axon_ntff_profile_hook = lambda h: setattr(_m, "_hook", h)
    _m.get_axon_ntff_profile_hook = lambda: _m._hook
    sys.modules["antenv.axon_hooks"] = _m
    try:
        import antenv

        antenv.axon_hooks = _m
        from trn_agent_boot.trn_boot import _ntff_profile_via_ctypes

        _m.set_axon_ntff_profile_hook(
            _ntff_profile_via_ctypes("/opt/axon/libaxon_pjrt.so")
        )
    except Exception:
        pass

import concourse.bass as bass
import concourse.mybir as mybir
import concourse.tile as tile
from concourse import bacc
from concourse import bass_utils

bass_utils.upload_artifacts = lambda tmpdir: tmpdir  # no fish bucket here
from concourse.bass_utils import run_bass_kernel_spmd

BF16 = mybir.dt.bfloat16
FP8 = mybir.dt.float8e4
F32 = mybir.dt.float32
AF = mybir.ActivationFunctionType
ALU = mybir.AluOpType
DR = mybir.MatmulPerfMode.DoubleRow

B, S, C = 2, 2048, 1024
NH, HD = 16, 64
SCALE = HD ** -0.5
NCORES = 8
R = (B * S) // NCORES        # 512 rows per core
P = 128
CH = C // P                  # 8 contraction chunks
RCH = R // P                 # 4 row chunks per core
HP = NH // 2                 # 8 head-pairs (one 128-partition chunk each)
W8S = 512.0                  # host pre-scale on fp8 W1/W2

_CACHE = {}


def _build(kv_bias: bool):
    """Build + compile the 8-core SPMD program. Returns the Bacc graph."""
    nc = bacc.Bacc("TRN2", target_bir_lowering=False, debug=False, num_devices=NCORES)

    # ---- DRAM I/O (per-core shapes; data differs per core) ----
    xtb_d = nc.dram_tensor("xtb", [P, CH * R], BF16, kind="ExternalInput")
    wq_d = nc.dram_tensor("wq", [P, CH * C], BF16, kind="ExternalInput")
    wk_d = nc.dram_tensor("wk", [P, CH * C], BF16, kind="ExternalInput")
    wv_d = nc.dram_tensor("wv", [P, CH * C], BF16, kind="ExternalInput")
    w1_d = nc.dram_tensor("w1", [P, CH * C], FP8, kind="ExternalInput")
    w2_d = nc.dram_tensor("w2", [P, CH * C], FP8, kind="ExternalInput")
    bqs_d = nc.dram_tensor("bqs", [P, CH], F32, kind="ExternalInput")
    b1r_d = nc.dram_tensor("b1r", [P, CH], F32, kind="ExternalInput")
    b2r_d = nc.dram_tensor("b2r", [P, CH], F32, kind="ExternalInput")
    if kv_bias:
        bkr_d = nc.dram_tensor("bkr", [1, C], BF16, kind="ExternalInput")
        bvr_d = nc.dram_tensor("bvr", [1, C], BF16, kind="ExternalInput")
    yt_d = nc.dram_tensor("yt", [P, CH * R], BF16, kind="ExternalOutput")

    # Internal DRAM for the two ktv AllGathers (one per head-half; each core
    # contributes its tightly-packed 64KB partial [64, 8 heads x 64], receives
    # the 4-core group's 256KB concat on the partition axis, sums shards
    # locally and expands into the block-diagonal layout).
    KF = (HP // 2) * HD          # 256: packed free size per half
    ktv_loc = [nc.dram_tensor(f"ktv_loc{i}", [P, KF], BF16) for i in (0, 1)]
    ktv_gth = [nc.dram_tensor(f"ktv_gth{i}", [4 * P, KF], BF16) for i in (0, 1)]
    groups = [[0, 1, 2, 3], [4, 5, 6, 7]]

    with tile.TileContext(nc) as tc:
        with (
            tc.tile_pool(name="persist", bufs=1) as pp,
            tc.tile_pool(name="ypool", bufs=3) as yp,
            tc.tile_pool(name="psum", bufs=6, space="PSUM") as psp,
            # W2 gets its own 2-bank pool so its 8 accumulation groups can't
            # all run ahead of the evictions (forces pipelined y evictions
            # instead of a serialized tail).
            tc.tile_pool(name="psum2", bufs=2, space="PSUM") as ps2,
        ):
            # ---- persistent SBUF tiles ----
            # xtb: chunk c at [:, c*R:(c+1)*R]. wk/wv: half-blocked layout,
            # (oh, c) slice at [:, oh*CH*512 + c*512 :+ 512] (matches the
            # host packing; enables large consolidated input DMAs).
            xtb_t = pp.tile([P, CH * R], BF16, name="xtb_t")
            wk_t = pp.tile([P, CH * C], BF16, name="wk_t")
            wv_t = pp.tile([P, CH * C], BF16, name="wv_t")
            xtb = [xtb_t[:, c * R : (c + 1) * R] for c in range(CH)]
            HB = CH * 512                # fh/sh block size in the wk/wv layout
            wq = pp.tile([P, CH * C], BF16, name="wq_sb")
            w1 = pp.tile([P, CH * C], FP8, name="w1_sb")
            w2 = pp.tile([P, CH * C], FP8, name="w2_sb")
            bqs = pp.tile([P, CH], F32, name="bqs_sb")
            b1r = pp.tile([P, CH], F32, name="b1r_sb")
            b2r = pp.tile([P, CH], F32, name="b2r_sb")
            k_sb = [pp.tile([P, C], BF16, name=f"k_sb{i}") for i in range(RCH)]
            v_sb = [pp.tile([P, C], BF16, name=f"v_sb{i}") for i in range(RCH)]
            q_sb = [pp.tile([P, R], BF16, name=f"q_sb{i}") for i in range(HP)]
            # ox[m] = out' + b2 + x' (residual pre-sum, built during out')
            ox = [pp.tile([P, R], BF16, name=f"ox{i}") for i in range(HP)]
            out8 = pp.tile([P, CH * R], FP8, name="out8_sb")
            h8 = pp.tile([P, CH * R], FP8, name="h8_sb")
            ktv_acc = [pp.tile([P, KF], BF16, name=f"ktv_acc{i}") for i in (0, 1)]
            gsb = [pp.tile([P, 4 * KF], BF16, name=f"gsb{i}") for i in (0, 1)]
            gt = [pp.tile([P, KF], BF16, name=f"gt{i}") for i in (0, 1)]
            ktv_bb = pp.tile([P, HP * P], BF16, name="ktv_bb")
            if kv_bias:
                ones = pp.tile([1, P], BF16, name="ones_sb")
                bkr = pp.tile([1, C], BF16, name="bkr_sb")
                bvr = pp.tile([1, C], BF16, name="bvr_sb")

            # ---- input DMAs (dual HWDGE queues, consolidated) ----
            # Small DMAs pay ~1KB/partition-row descriptors (~190 GB/s); big
            # spans get 4-8KB descriptors (~340 GB/s). Two parallel queue
            # rows: sync carries wk/wq/w1/w2 (+readbacks), scalar carries
            # xtb/wv (+ktv bounce + y out). First chunks ship small so
            # compute starts immediately. wk/wv DRAM layout is half-blocked:
            # fh chunks 0-7, then sh chunks 0-7.
            for c in range(4):
                nc.scalar.dma_start(
                    out=xtb_t[:, c * R : (c + 1) * R],
                    in_=xtb_d[:, c * R : (c + 1) * R],
                )
                nc.sync.dma_start(
                    out=wk_t[:, c * 512 : (c + 1) * 512],
                    in_=wk_d[:, c * 512 : (c + 1) * 512],
                )
            nc.scalar.dma_start(
                out=xtb_t[:, 4 * R : CH * R], in_=xtb_d[:, 4 * R : CH * R]
            )
            nc.sync.dma_start(
                out=wk_t[:, 4 * 512 : HB], in_=wk_d[:, 4 * 512 : HB]
            )
            nc.scalar.dma_start(out=wv_t[:, 0:HB], in_=wv_d[:, 0:HB])
            nc.sync.dma_start(out=wk_t[:, HB : 2 * HB], in_=wk_d[:, HB : 2 * HB])
            nc.scalar.dma_start(out=wv_t[:, HB : 2 * HB], in_=wv_d[:, HB : 2 * HB])
            if kv_bias:
                nc.vector.memset(ones[:], 1.0)
                nc.sync.dma_start(out=bkr[:], in_=bkr_d[:])
                nc.sync.dma_start(out=bvr[:], in_=bvr_d[:])
            nc.sync.dma_start(out=wq[:], in_=wq_d[:])
            nc.sync.dma_start(out=bqs[:], in_=bqs_d[:])
            nc.sync.dma_start(out=w1[:], in_=w1_d[:])
            nc.sync.dma_start(out=b1r[:], in_=b1r_d[:])
            nc.sync.dma_start(out=w2[:], in_=w2_d[:])
            nc.sync.dma_start(out=b2r[:], in_=b2r_d[:])
            # zero the ktv block-diagonal tile (only head-diagonal blocks are
            # filled from the gathered sum)
            nc.vector.memset(ktv_bb[:], 0.0)

            # ---- k, v projections (row-major [r, o]) ----
            # contraction-OUTER loops, split by output half (oh): compute
            # k(oh) then v(oh), then the 4 head-pair ktv blocks of that half,
            # and launch that half's AllGather immediately.
            def proj_kv(w_t, brow, dst, oh):
                pss = [
                    psp.tile([P, 512], F32, name="ps", tag="ps")
                    for _ in range(RCH)
                ]
                for c in range(CH):
                    for ri in range(RCH):
                        nc.tensor.matmul(
                            pss[ri][:],
                            xtb_t[:, c * R + ri * P : c * R + (ri + 1) * P],
                            w_t[:, oh * HB + c * 512 : oh * HB + (c + 1) * 512],
                            start=(c == 0),
                            stop=(c == CH - 1 and not kv_bias),
                        )
                for ri in range(RCH):
                    ps = pss[ri]
                    if kv_bias:
                        nc.tensor.matmul(
                            ps[:],
                            ones[:1, :],
                            brow[:1, oh * 512 : (oh + 1) * 512],
                            start=False,
                            stop=True,
                        )
                    dst_ap = dst[ri][:, oh * 512 : (oh + 1) * 512]
                    if ri % 2 == 0:
                        nc.vector.tensor_copy(dst_ap, ps[:])
                    else:
                        nc.scalar.activation(dst_ap, ps[:], AF.Copy)

            for oh in range(2):
                proj_kv(wk_t, bkr if kv_bias else None, k_sb, oh)
                proj_kv(wv_t, bvr if kv_bias else None, v_sb, oh)

                # partial ktv for this half: head-pairs packed [128,128];
                # diagonal blocks are the per-head ktvs, off-diagonal is
                # garbage. Pack the diagonal strips tightly ([128, 4x64]:
                # even heads on partitions 0-63, odd heads on 64-127) and
                # trigger the AllGather.
                with tc.high_priority(offset=400):
                    pk = psp.tile([P, 512], F32, name="ps", tag="ps")
                    for hpl in range(HP // 2):
                        hp = oh * (HP // 2) + hpl
                        for ri in range(RCH):
                            nc.tensor.matmul(
                                pk[:, hpl * P : (hpl + 1) * P],
                                k_sb[ri][:, hp * P : (hp + 1) * P],
                                v_sb[ri][:, hp * P : (hp + 1) * P],
                                start=(ri == 0),
                                stop=(ri == RCH - 1),
                            )
                    pk_v = pk.rearrange("p (hp t d) -> p hp t d", hp=HP // 2, t=2, d=HD)
                    acc_v = ktv_acc[oh].rearrange(
                        "p (hp d) -> p hp d", hp=HP // 2, d=HD
                    )
                    nc.vector.tensor_copy(
                        acc_v[0:HD, :, :], pk_v[0:HD, :, 0, :]
                    )
                    nc.vector.tensor_copy(
                        acc_v[HD:P, :, :], pk_v[HD:P, :, 1, :]
                    )
                    nc.scalar.dma_start(out=ktv_loc[oh][:], in_=ktv_acc[oh][:])
                    nc.gpsimd.collective_compute(
                        "AllGather",
                        ALU.bypass,
                        replica_groups=groups,
                        ins=[ktv_loc[oh][:]],
                        outs=[ktv_gth[oh][:]],
                    )

            # ---- q' projection (feature-major [o, r]) ----
            # All collective-dependent work is ordered after it.
            for m in range(CH):
                ps = psp.tile([P, 512], F32, name="ps", tag="ps")
                for c in range(CH):
                    nc.tensor.matmul(
                        ps[:],
                        wq[:, c * C + m * P : c * C + (m + 1) * P],
                        xtb_t[:, c * R : (c + 1) * R],
                        start=(c == 0),
                        stop=(c == CH - 1),
                    )
                # q evictions stay on ACT: the DVE FIFO must remain clear for
                # the gather-sum adds that gate out' right after q finishes
                nc.scalar.activation(
                    q_sb[m][:], ps[:], AF.Identity, bias=bqs[:, m : m + 1]
                )

            # ---- AllGather readback, shard sum + block-diag expansion ----
            def gather_sum(oh):
                with tc.high_priority(offset=300):
                    # single strided DMA: DRAM [4*P, KF] -> SBUF [P, 4*KF]
                    nc.scalar.dma_start(
                        out=gsb[oh].rearrange("p (r f) -> p r f", r=4),
                        in_=ktv_gth[oh].rearrange("(r p) f -> p r f", r=4),
                    )
                    g = gsb[oh]
                    nc.vector.tensor_add(
                        gt[oh][:], g[:, 0:KF], g[:, KF : 2 * KF]
                    )
                    nc.vector.tensor_add(
                        g[:, 0:KF], g[:, 2 * KF : 3 * KF], g[:, 3 * KF : 4 * KF]
                    )
                    # final add writes straight into the zeroed block-diagonal
                    # layout: even heads -> partitions 0-63 of t=0 blocks,
                    # odd heads -> partitions 64-127 of t=1 blocks.
                    bb_v = ktv_bb[:, oh * 512 : (oh + 1) * 512].rearrange(
                        "p (hp t d) -> p hp t d", hp=HP // 2, t=2, d=HD
                    )
                    gt_v = gt[oh].rearrange("p (hp d) -> p hp d", hp=HP // 2, d=HD)
                    g0_v = g[:, 0:KF].rearrange("p (hp d) -> p hp d", hp=HP // 2, d=HD)
                    nc.vector.tensor_add(
                        bb_v[0:HD, :, 0, :], gt_v[0:HD, :, :], g0_v[0:HD, :, :]
                    )
                    nc.vector.tensor_add(
                        bb_v[HD:P, :, 1, :], gt_v[HD:P, :, :], g0_v[HD:P, :, :]
                    )

            # ---- out' = blockdiag(ktv).T @ q' + fp8 MLP ----
            HPH = HP // 2

            def out_chunk(hp):
                ps = psp.tile([P, 512], F32, name="ps", tag="ps")
                nc.tensor.matmul(
                    ps[:],
                    ktv_bb[:, hp * P : (hp + 1) * P],
                    q_sb[hp][:],
                    start=True,
                    stop=True,
                )
                # ox = (out' + b2) + x' -- the full non-MLP part of y, so the
                # final y eviction is a single DVE op per chunk. out8 (the
                # fp8 MLP operand) alternates engines: it gates the W1
                # matmuls, so halving its eviction latency matters.
                nc.vector.scalar_tensor_tensor(
                    ox[hp][:], ps[:], b2r[:, hp : hp + 1],
                    xtb_t[:, hp * R : (hp + 1) * R], ALU.add, ALU.add,
                )
                if hp % 2 == 0:
                    nc.scalar.activation(
                        out8[:, hp * R : (hp + 1) * R], ps[:], AF.Copy
                    )
                else:
                    nc.vector.tensor_copy(out8[:, hp * R : (hp + 1) * R], ps[:])

            def w1v(cp, j):
                sl = w1[:, cp * 2 * C : (cp + 1) * 2 * C]
                return sl.rearrange("p (t f) -> p t f", t=2)[
                    :, :, j * P : (j + 1) * P
                ]

            def o8v(cp):
                sl = out8[:, cp * 2 * R : (cp + 1) * 2 * R]
                return sl.rearrange("p (t f) -> p t f", t=2)

            gather_sum(0)
            for hp in range(HPH):
                out_chunk(hp)
            # h' partial: j-groups 0-4 over the available c-pairs 0,1
            # (5 held PSUM groups; the 6-bank pool keeps one rotating)
            NHLD = 5
            hps = []
            for j in range(NHLD):
                ps = psp.tile([P, 512], F32, name="ps", tag="ps")
                hps.append(ps)
                for cp in range(2):
                    nc.tensor.matmul(
                        ps[:], w1v(cp, j), o8v(cp),
                        start=(cp == 0), stop=False, perf_mode=DR,
                    )
            gather_sum(1)
            for hp in range(HPH, HP):
                out_chunk(hp)

            # ---- finish h' = gelu((W1*512) out' /512 + b1) -> fp8 ----
            for j in range(NHLD):
                ps = hps[j]
                for cp in range(2, 4):
                    nc.tensor.matmul(
                        ps[:], w1v(cp, j), o8v(cp),
                        start=False, stop=(cp == 3), perf_mode=DR,
                    )
                nc.scalar.activation(
                    h8[:, j * R : (j + 1) * R], ps[:], AF.Gelu,
                    bias=b1r[:, j : j + 1], scale=1.0 / W8S,
                )
            for j in range(NHLD, CH):
                ps = psp.tile([P, 512], F32, name="ps", tag="ps")
                for cp in range(4):
                    nc.tensor.matmul(
                        ps[:], w1v(cp, j), o8v(cp),
                        start=(cp == 0), stop=(cp == 3), perf_mode=DR,
                    )
                nc.scalar.activation(
                    h8[:, j * R : (j + 1) * R], ps[:], AF.Gelu,
                    bias=b1r[:, j : j + 1], scale=1.0 / W8S,
                )

            # ---- MLP out + residual: y' = (W2 h')/512 + b2 + out' + x' ----
            def w2v(jp, m):
                sl = w2[:, jp * 2 * C : (jp + 1) * 2 * C]
                return sl.rearrange("p (t f) -> p t f", t=2)[
                    :, :, m * P : (m + 1) * P
                ]

            def h8v(jp):
                sl = h8[:, jp * 2 * R : (jp + 1) * 2 * R]
                return sl.rearrange("p (t f) -> p t f", t=2)

            for m in range(CH):
                ps = ps2.tile([P, 512], F32, name="ps2", tag="ps2")
                for jp in range(4):
                    nc.tensor.matmul(
                        ps[:], w2v(jp, m), h8v(jp),
                        start=(jp == 0), stop=(jp == 3), perf_mode=DR,
                    )
                # y = psum/512 + (out' + b2 + x'): one DVE op per chunk. The
                # last chunk's eviction + DMA is the kernel tail -- split it
                # in half so the store starts sooner.
                y_t = yp.tile([P, 512], BF16, name="y_t")
                if m < CH - 1:
                    nc.vector.scalar_tensor_tensor(
                        y_t[:], ps[:], 1.0 / W8S, ox[m][:], ALU.mult, ALU.add
                    )
                    nc.scalar.dma_start(out=yt_d[:, m * R : (m + 1) * R], in_=y_t[:])
                else:
                    for h in range(2):
                        sl = slice(h * 256, (h + 1) * 256)
                        nc.vector.scalar_tensor_tensor(
                            y_t[:, sl], ps[:, sl], 1.0 / W8S, ox[m][:, sl],
                            ALU.mult, ALU.add,
                        )
                        nc.scalar.dma_start(
                            out=yt_d[:, m * R + h * 256 : m * R + (h + 1) * 256],
                            in_=y_t[:, sl],
                        )

    nc.compile()
    return nc


def _get_nc(kv_bias: bool):
    key = ("nc", kv_bias)
    if key not in _CACHE:
        _CACHE[key] = _build(kv_bias)
    return _CACHE[key]


def _pack_pf(a):
    """[CH*P, F] row-major -> [P, CH*F] (partition-chunk packing)."""
    n, f = a.shape
    ch = n // P
    return np.ascontiguousarray(a.reshape(ch, P, f).transpose(1, 0, 2).reshape(P, ch * f))


def _prep_inputs(x, Wq, bq, Wk, bk, Wv, bv, W1, b1, W2, b2, kv_bias):
    bf = ml_dtypes.bfloat16
    f8 = ml_dtypes.float8_e4m3
    def _half_block(a):
        # [P, CH*C] chunk-major -> [P, (2, CH, 512)] half-blocked
        return np.ascontiguousarray(
            a.reshape(P, CH, 2, 512).transpose(0, 2, 1, 3).reshape(P, CH * C)
        )

    wq_p = _pack_pf((Wq.T * SCALE).astype(np.float32)).astype(bf)
    wk_p = _half_block(_pack_pf(np.ascontiguousarray(Wk.T))).astype(bf)
    wv_p = _half_block(_pack_pf(np.ascontiguousarray(Wv.T))).astype(bf)
    w1_p = np.clip(_pack_pf(np.ascontiguousarray(W1.T)) * W8S, -240, 240).astype(f8)
    w2_p = np.clip(_pack_pf(np.ascontiguousarray(W2.T)) * W8S, -240, 240).astype(f8)
    bqs = np.ascontiguousarray((bq * SCALE).astype(np.float32).reshape(CH, P).T)
    b1r = np.ascontiguousarray(b1.astype(np.float32).reshape(CH, P).T)
    b2r = np.ascontiguousarray(b2.astype(np.float32).reshape(CH, P).T)

    xf = x.reshape(B * S, C)
    in_maps = []
    for core in range(NCORES):
        xs = xf[core * R : (core + 1) * R]           # [R, C]
        xt = _pack_pf(np.ascontiguousarray(xs.T))    # [P, CH*R] f32
        m = {
            "xtb": xt.astype(bf),
            "wq": wq_p,
            "wk": wk_p,
            "wv": wv_p,
            "w1": w1_p,
            "w2": w2_p,
            "bqs": bqs,
            "b1r": b1r,
            "b2r": b2r,
        }
        if kv_bias:
            m["bkr"] = bk.astype(bf).reshape(1, C)
            m["bvr"] = bv.astype(bf).reshape(1, C)
        in_maps.append(m)
    return in_maps


def _unpack_out(results):
    y = np.empty((B * S, C), np.float32)
    for core in range(NCORES):
        yt = np.asarray(results[core]["yt"], dtype=np.float32)   # [P, CH*R]
        blk = yt.reshape(P, CH, R).transpose(1, 0, 2).reshape(C, R)
        y[core * R : (core + 1) * R] = blk.T
    return y.reshape(B, S, C)


def _run(inputs, trace=False, trace_cores=None):
    x = np.asarray(inputs["x"], np.float32)
    args = [np.asarray(inputs[k], np.float32) for k in
            ("Wq", "bq", "Wk", "bk", "Wv", "bv", "W1", "b1", "W2", "b2")]
    kv_bias = bool(np.any(args[3]) or np.any(args[5]))
    nc = _get_nc(kv_bias)
    in_maps = _prep_inputs(x, *args, kv_bias)
    res = run_bass_kernel_spmd(
        nc, in_maps, core_ids=list(range(NCORES)), trace=trace,
        trace_cores=trace_cores,
    )
    return _unpack_out(res.results), res


def kernel(**inputs) -> np.ndarray:
    out, _ = _run(inputs, trace=False)
    return out


def kernel_profiled(**inputs):
    """Returns (output, exec_time_ns) using neuron-profile NTFF timing."""
    out, res = _run(inputs, trace=True)
    return out, res.exec_time_ns


# revision 36
# speedup vs baseline: 1.1333x; 1.0981x over previous
"""Trainium2 Bass kernel: dense transformer block (bilinear attention, no softmax).

Reference computation (B=2, S=2048, C=1024, H=16 heads, hd=64, HIDDEN=1024):
    q = split_heads(x @ Wq.T + bq) * hd**-0.5
    k = split_heads(x @ Wk.T + bk)
    v = split_heads(x @ Wv.T + bv)
    out = (q @ k.T) @ v          per (batch, head)   <-- no softmax!
    h = gelu(out @ W1.T + b1);  mlp = h @ W2.T + b2
    y = x + out + mlp

Key algebraic optimization: (q @ k.T) @ v == q @ (k.T @ v). k.T@v is a tiny
[64,64] per head, so attention drops from ~34 GFLOP to ~1 GFLOP.

Sharding (8 cores): rows (batch*seq = 4096) split 512/core; cores 0-3 hold
batch 0, cores 4-7 batch 1. Each core computes q/k/v/MLP for its rows only.
The only cross-core dependency is ktv = k.T@v (contraction over the full 2048
rows of a batch). Each core computes its partial ktv; two 128KB AllGathers
per 4-core batch group (split by head-halves, triggered as soon as each half
of k/v is done) exchange the partials, and the DVE sums the four shards
locally -- AllGather has a much lower ncfw floor than AllReduce and the ~1us
of DVE adds is free. All collective-dependent compute (out', MLP) is ordered
AFTER the whole q projection so the PE has maximal independent work to
overlap the collectives' latency + the PJRT core-start stagger.

Precision: attention path (q/k/v/ktv/out) runs bf16 with fp32 PSUM. The MLP
runs fp8 e4m3 with DoubleRow matmuls (contraction 256/instr, ~1.8x faster):
out and h are evicted as fp8 (scale 1), W1/W2 are pre-scaled x512 on host
(fp8 exponent range) and the 1/512 is folded into the PSUM-eviction scale.
Measured accuracy ~1.3e-2 absmax rel vs the fp32 reference (limit 2e-2).
Output y is written bf16 and cast to f32 on host.
"""

import sys
import types

sys.path.insert(0, "/opt/trn_rl_repo")

import numpy as np
import ml_dtypes

# ---------------------------------------------------------------------------
# NTFF profile hook shim (this image's antenv lacks axon_hooks; inject it so
# run_bass_kernel_spmd(trace=True) can profile). Harmless when unused.
# ---------------------------------------------------------------------------
if "antenv.axon_hooks" not in sys.modules:
    _m = types.ModuleType("antenv.axon_hooks")
    _m._hook = None
    _m.set_axon_ntff_profile_hook = lambda h: setattr(_m, "_hook", h)
    _m.get_axon_ntff_profile_hook = lambda: _m._hook
    sys.modules["antenv.axon_hooks"] = _m
    try:
        import antenv

        antenv.axon_hooks = _m
        from trn_agent_boot.trn_boot import _ntff_profile_via_ctypes

        _m.set_axon_ntff_profile_hook(
            _ntff_profile_via_ctypes("/opt/axon/libaxon_pjrt.so")
        )
    except Exception:
        pass

import concourse.bass as bass
import concourse.mybir as mybir
import concourse.tile as tile
from concourse import bacc
from concourse import bass_utils

bass_utils.upload_artifacts = lambda tmpdir: tmpdir  # no fish bucket here
from concourse.bass_utils import run_bass_kernel_spmd

BF16 = mybir.dt.bfloat16
FP8 = mybir.dt.float8e4
F32 = mybir.dt.float32
AF = mybir.ActivationFunctionType
ALU = mybir.AluOpType
DR = mybir.MatmulPerfMode.DoubleRow

B, S, C = 2, 2048, 1024
NH, HD = 16, 64
SCALE = HD ** -0.5
NCORES = 8
R = (B * S) // NCORES        # 512 rows per core
P = 128
CH = C // P                  # 8 contraction chunks
RCH = R // P                 # 4 row chunks per core
HP = NH // 2                 # 8 head-pairs (one 128-partition chunk each)
W8S = 512.0                  # host pre-scale on fp8 W1/W2

_CACHE = {}


def _build(kv_bias: bool):
    """Build + compile the 8-core SPMD program. Returns the Bacc graph."""
    nc = bacc.Bacc("TRN2", target_bir_lowering=False, debug=False, num_devices=NCORES)

    # ---- DRAM I/O (per-core shapes; data differs per core) ----
    xtb_d = nc.dram_tensor("xtb", [P, CH * R], BF16, kind="ExternalInput")
    wq_d = nc.dram_tensor("wq", [P, CH * C], BF16, kind="ExternalInput")
    wk_d = nc.dram_tensor("wk", [P, CH * C], BF16, kind="ExternalInput")
    wv_d = nc.dram_tensor("wv", [P, CH * C], BF16, kind="ExternalInput")
    w1_d = nc.dram_tensor("w1", [P, CH * C], FP8, kind="ExternalInput")
    w2_d = nc.dram_tensor("w2", [P, CH * C], FP8, kind="ExternalInput")
    bqs_d = nc.dram_tensor("bqs", [P, CH], F32, kind="ExternalInput")
    b1r_d = nc.dram_tensor("b1r", [P, CH], F32, kind="ExternalInput")
    b2r_d = nc.dram_tensor("b2r", [P, CH], F32, kind="ExternalInput")
    if kv_bias:
        bkr_d = nc.dram_tensor("bkr", [1, C], BF16, kind="ExternalInput")
        bvr_d = nc.dram_tensor("bvr", [1, C], BF16, kind="ExternalInput")
    yt_d = nc.dram_tensor("yt", [P, CH * R], BF16, kind="ExternalOutput")

    # Internal DRAM for the two ktv AllGathers (one per head-half; each core
    # contributes its tightly-packed 64KB partial [64, 8 heads x 64], receives
    # the 4-core group's 256KB concat on the partition axis, sums shards
    # locally and expands into the block-diagonal layout).
    KF = (HP // 2) * HD          # 256: packed free size per half
    ktv_loc = [nc.dram_tensor(f"ktv_loc{i}", [P, KF], BF16) for i in (0, 1)]
    ktv_gth = [nc.dram_tensor(f"ktv_gth{i}", [4 * P, KF], BF16) for i in (0, 1)]
    groups = [[0, 1, 2, 3], [4, 5, 6, 7]]

    with tile.TileContext(nc) as tc:
        with (
            tc.tile_pool(name="persist", bufs=1) as pp,
            tc.tile_pool(name="ypool", bufs=3) as yp,
            tc.tile_pool(name="psum", bufs=6, space="PSUM") as psp,
            # W2 gets its own 2-bank pool so its 8 accumulation groups can't
            # all run ahead of the evictions (forces pipelined y evictions
            # instead of a serialized tail).
            tc.tile_pool(name="psum2", bufs=2, space="PSUM") as ps2,
        ):
            # ---- persistent SBUF tiles ----
            # xtb: chunk c at [:, c*R:(c+1)*R]. wk/wv: half-blocked layout,
            # (oh, c) slice at [:, oh*CH*512 + c*512 :+ 512] (matches the
            # host packing; enables large consolidated input DMAs).
            xtb_t = pp.tile([P, CH * R], BF16, name="xtb_t")
            wk_t = pp.tile([P, CH * C], BF16, name="wk_t")
            wv_t = pp.tile([P, CH * C], BF16, name="wv_t")
            xtb = [xtb_t[:, c * R : (c + 1) * R] for c in range(CH)]
            HB = CH * 512                # fh/sh block size in the wk/wv layout
            wq = pp.tile([P, CH * C], BF16, name="wq_sb")
            w1 = pp.tile([P, CH * C], FP8, name="w1_sb")
            w2 = pp.tile([P, CH * C], FP8, name="w2_sb")
            bqs = pp.tile([P, CH], F32, name="bqs_sb")
            b1r = pp.tile([P, CH], F32, name="b1r_sb")
            b2r = pp.tile([P, CH], F32, name="b2r_sb")
            k_sb = [pp.tile([P, C], BF16, name=f"k_sb{i}") for i in range(RCH)]
            v_sb = [pp.tile([P, C], BF16, name=f"v_sb{i}") for i in range(RCH)]
            q_sb = [pp.tile([P, R], BF16, name=f"q_sb{i}") for i in range(HP)]
            # ox[m] = out' + b2 + x' (residual pre-sum, built during out')
            ox = [pp.tile([P, R], BF16, name=f"ox{i}") for i in range(HP)]
            out8 = pp.tile([P, CH * R], FP8, name="out8_sb")
            h8 = pp.tile([P, CH * R], FP8, name="h8_sb")
            ktv_acc = [pp.tile([P, KF], BF16, name=f"ktv_acc{i}") for i in (0, 1)]
            gsb = [pp.tile([P, 4 * KF], BF16, name=f"gsb{i}") for i in (0, 1)]
            gt = [pp.tile([P, KF], BF16, name=f"gt{i}") for i in (0, 1)]
            ktv_bb = pp.tile([P, HP * P], BF16, name="ktv_bb")
            if kv_bias:
                ones = pp.tile([1, P], BF16, name="ones_sb")
                bkr = pp.tile([1, C], BF16, name="bkr_sb")
                bvr = pp.tile([1, C], BF16, name="bvr_sb")

            # ---- input DMAs (dual HWDGE queues, consolidated) ----
            # Small DMAs pay ~1KB/partition-row descriptors (~190 GB/s); big
            # spans get 4-8KB descriptors (~340 GB/s). Two parallel queue
            # rows: sync carries wk/wq/w1/w2 (+readbacks), scalar carries
            # xtb/wv (+ktv bounce + y out). First chunks ship small so
            # compute starts immediately. wk/wv DRAM layout is half-blocked:
            # fh chunks 0-7, then sh chunks 0-7.
            for c in range(4):
                nc.scalar.dma_start(
                    out=xtb_t[:, c * R : (c + 1) * R],
                    in_=xtb_d[:, c * R : (c + 1) * R],
                )
                nc.sync.dma_start(
                    out=wk_t[:, c * 512 : (c + 1) * 512],
                    in_=wk_d[:, c * 512 : (c + 1) * 512],
                )
            nc.scalar.dma_start(
                out=xtb_t[:, 4 * R : CH * R], in_=xtb_d[:, 4 * R : CH * R]
            )
            nc.sync.dma_start(
                out=wk_t[:, 4 * 512 : HB], in_=wk_d[:, 4 * 512 : HB]
            )
            nc.scalar.dma_start(out=wv_t[:, 0:HB], in_=wv_d[:, 0:HB])
            nc.sync.dma_start(out=wk_t[:, HB : 2 * HB], in_=wk_d[:, HB : 2 * HB])
            nc.scalar.dma_start(out=wv_t[:, HB : 2 * HB], in_=wv_d[:, HB : 2 * HB])
            if kv_bias:
                nc.vector.memset(ones[:], 1.0)
                nc.sync.dma_start(out=bkr[:], in_=bkr_d[:])
                nc.sync.dma_start(out=bvr[:], in_=bvr_d[:])
            nc.sync.dma_start(out=wq[:], in_=wq_d[:])
            nc.sync.dma_start(out=bqs[:], in_=bqs_d[:])
            nc.sync.dma_start(out=w1[:], in_=w1_d[:])
            nc.sync.dma_start(out=b1r[:], in_=b1r_d[:])
            nc.sync.dma_start(out=w2[:], in_=w2_d[:])
            nc.sync.dma_start(out=b2r[:], in_=b2r_d[:])
            # zero the ktv block-diagonal tile (only head-diagonal blocks are
            # filled from the gathered sum)
            nc.vector.memset(ktv_bb[:], 0.0)

            # ---- k, v projections (row-major [r, o]) ----
            # contraction-OUTER loops, split by output half (oh): compute
            # k(oh) then v(oh), then the 4 head-pair ktv blocks of that half,
            # and launch that half's AllGather immediately.
            def proj_kv(w_t, brow, dst, oh):
                pss = [
                    psp.tile([P, 512], F32, name="ps", tag="ps")
                    for _ in range(RCH)
                ]
                for c in range(CH):
                    for ri in range(RCH):
                        nc.tensor.matmul(
                            pss[ri][:],
                            xtb_t[:, c * R + ri * P : c * R + (ri + 1) * P],
                            w_t[:, oh * HB + c * 512 : oh * HB + (c + 1) * 512],
                            start=(c == 0),
                            stop=(c == CH - 1 and not kv_bias),
                        )
                for ri in range(RCH):
                    ps = pss[ri]
                    if kv_bias:
                        nc.tensor.matmul(
                            ps[:],
                            ones[:1, :],
                            brow[:1, oh * 512 : (oh + 1) * 512],
                            start=False,
                            stop=True,
                        )
                    dst_ap = dst[ri][:, oh * 512 : (oh + 1) * 512]
                    if ri % 2 == 0:
                        nc.vector.tensor_copy(dst_ap, ps[:])
                    else:
                        nc.scalar.activation(dst_ap, ps[:], AF.Copy)

            for oh in range(2):
                proj_kv(wk_t, bkr if kv_bias else None, k_sb, oh)
                proj_kv(wv_t, bvr if kv_bias else None, v_sb, oh)

                # partial ktv for this half: head-pairs packed [128,128];
                # diagonal blocks are the per-head ktvs, off-diagonal is
                # garbage. Pack the diagonal strips tightly ([128, 4x64]:
                # even heads on partitions 0-63, odd heads on 64-127) and
                # trigger the AllGather.
                with tc.high_priority(offset=400):
                    pk = psp.tile([P, 512], F32, name="ps", tag="ps")
                    for hpl in range(HP // 2):
                        hp = oh * (HP // 2) + hpl
                        for ri in range(RCH):
                            nc.tensor.matmul(
                                pk[:, hpl * P : (hpl + 1) * P],
                                k_sb[ri][:, hp * P : (hp + 1) * P],
                                v_sb[ri][:, hp * P : (hp + 1) * P],
                                start=(ri == 0),
                                stop=(ri == RCH - 1),
                            )
                    pk_v = pk.rearrange("p (hp t d) -> p hp t d", hp=HP // 2, t=2, d=HD)
                    acc_v = ktv_acc[oh].rearrange(
                        "p (hp d) -> p hp d", hp=HP // 2, d=HD
                    )
                    nc.vector.tensor_copy(
                        acc_v[0:HD, :, :], pk_v[0:HD, :, 0, :]
                    )
                    nc.vector.tensor_copy(
                        acc_v[HD:P, :, :], pk_v[HD:P, :, 1, :]
                    )
                    nc.scalar.dma_start(out=ktv_loc[oh][:], in_=ktv_acc[oh][:])
                    nc.gpsimd.collective_compute(
                        "AllGather",
                        ALU.bypass,
                        replica_groups=groups,
                        ins=[ktv_loc[oh][:]],
                        outs=[ktv_gth[oh][:]],
                    )

            # ---- q' projection (feature-major [o, r]) ----
            # All collective-dependent work is ordered after it.
            for m in range(CH):
                ps = psp.tile([P, 512], F32, name="ps", tag="ps")
                for c in range(CH):
                    nc.tensor.matmul(
                        ps[:],
                        wq[:, c * C + m * P : c * C + (m + 1) * P],
                        xtb_t[:, c * R : (c + 1) * R],
                        start=(c == 0),
                        stop=(c == CH - 1),
                    )
                # q evictions stay on ACT: the DVE FIFO must remain clear for
                # the gather-sum adds that gate out' right after q finishes
                nc.scalar.activation(
                    q_sb[m][:], ps[:], AF.Identity, bias=bqs[:, m : m + 1]
                )

            # ---- AllGather readback, shard sum + block-diag expansion ----
            def gather_sum(oh):
                with tc.high_priority(offset=300):
                    # single strided DMA: DRAM [4*P, KF] -> SBUF [P, 4*KF].
                    # On sync: the ACT queue is busy with q evictions at this
                    # point, and engine FIFOs ignore priority.
                    nc.sync.dma_start(
                        out=gsb[oh].rearrange("p (r f) -> p r f", r=4),
                        in_=ktv_gth[oh].rearrange("(r p) f -> p r f", r=4),
                    )
                    g = gsb[oh]
                    nc.vector.tensor_add(
                        gt[oh][:], g[:, 0:KF], g[:, KF : 2 * KF]
                    )
                    nc.vector.tensor_add(
                        g[:, 0:KF], g[:, 2 * KF : 3 * KF], g[:, 3 * KF : 4 * KF]
                    )
                    # final add writes straight into the zeroed block-diagonal
                    # layout: even heads -> partitions 0-63 of t=0 blocks,
                    # odd heads -> partitions 64-127 of t=1 blocks.
                    bb_v = ktv_bb[:, oh * 512 : (oh + 1) * 512].rearrange(
                        "p (hp t d) -> p hp t d", hp=HP // 2, t=2, d=HD
                    )
                    gt_v = gt[oh].rearrange("p (hp d) -> p hp d", hp=HP // 2, d=HD)
                    g0_v = g[:, 0:KF].rearrange("p (hp d) -> p hp d", hp=HP // 2, d=HD)
                    nc.vector.tensor_add(
                        bb_v[0:HD, :, 0, :], gt_v[0:HD, :, :], g0_v[0:HD, :, :]
                    )
                    nc.vector.tensor_add(
                        bb_v[HD:P, :, 1, :], gt_v[HD:P, :, :], g0_v[HD:P, :, :]
                    )

            # ---- out' = blockdiag(ktv).T @ q' + fp8 MLP ----
            HPH = HP // 2

            def out_chunk(hp):
                ps = psp.tile([P, 512], F32, name="ps", tag="ps")
                nc.tensor.matmul(
                    ps[:],
                    ktv_bb[:, hp * P : (hp + 1) * P],
                    q_sb[hp][:],
                    start=True,
                    stop=True,
                )
                # ox = (out' + b2) + x' -- the full non-MLP part of y, so the
                # final y eviction is a single DVE op per chunk. out8 (the
                # fp8 MLP operand) alternates engines: it gates the W1
                # matmuls, so halving its eviction latency matters.
                nc.vector.scalar_tensor_tensor(
                    ox[hp][:], ps[:], b2r[:, hp : hp + 1],
                    xtb_t[:, hp * R : (hp + 1) * R], ALU.add, ALU.add,
                )
                if hp % 2 == 0:
                    nc.scalar.activation(
                        out8[:, hp * R : (hp + 1) * R], ps[:], AF.Copy
                    )
                else:
                    nc.vector.tensor_copy(out8[:, hp * R : (hp + 1) * R], ps[:])

            def w1v(cp, j):
                sl = w1[:, cp * 2 * C : (cp + 1) * 2 * C]
                return sl.rearrange("p (t f) -> p t f", t=2)[
                    :, :, j * P : (j + 1) * P
                ]

            def o8v(cp):
                sl = out8[:, cp * 2 * R : (cp + 1) * 2 * R]
                return sl.rearrange("p (t f) -> p t f", t=2)

            gather_sum(0)
            for hp in range(HPH):
                out_chunk(hp)
            # h' partial: j-groups 0-3 over the available c-pairs 0,1
            # (4 held PSUM groups; the 6-bank pool keeps two rotating so the
            # out' chunks pipeline their evictions)
            NHLD = 4
            hps = []
            for j in range(NHLD):
                ps = psp.tile([P, 512], F32, name="ps", tag="ps")
                hps.append(ps)
                for cp in range(2):
                    nc.tensor.matmul(
                        ps[:], w1v(cp, j), o8v(cp),
                        start=(cp == 0), stop=False, perf_mode=DR,
                    )
            gather_sum(1)
            for hp in range(HPH, HP):
                out_chunk(hp)

            # ---- finish h' = gelu((W1*512) out' /512 + b1) -> fp8 ----
            for j in range(NHLD):
                ps = hps[j]
                for cp in range(2, 4):
                    nc.tensor.matmul(
                        ps[:], w1v(cp, j), o8v(cp),
                        start=False, stop=(cp == 3), perf_mode=DR,
                    )
                nc.scalar.activation(
                    h8[:, j * R : (j + 1) * R], ps[:], AF.Gelu,
                    bias=b1r[:, j : j + 1], scale=1.0 / W8S,
                )
            for j in range(NHLD, CH):
                ps = psp.tile([P, 512], F32, name="ps", tag="ps")
                for cp in range(4):
                    nc.tensor.matmul(
                        ps[:], w1v(cp, j), o8v(cp),
                        start=(cp == 0), stop=(cp == 3), perf_mode=DR,
                    )
                nc.scalar.activation(
                    h8[:, j * R : (j + 1) * R], ps[:], AF.Gelu,
                    bias=b1r[:, j : j + 1], scale=1.0 / W8S,
                )

            # ---- MLP out + residual: y' = (W2 h')/512 + b2 + out' + x' ----
            def w2v(jp, m):
                sl = w2[:, jp * 2 * C : (jp + 1) * 2 * C]
                return sl.rearrange("p (t f) -> p t f", t=2)[
                    :, :, m * P : (m + 1) * P
                ]

            def h8v(jp):
                sl = h8[:, jp * 2 * R : (jp + 1) * 2 * R]
                return sl.rearrange("p (t f) -> p t f", t=2)

            for m in range(CH):
                ps = ps2.tile([P, 512], F32, name="ps2", tag="ps2")
                for jp in range(4):
                    nc.tensor.matmul(
                        ps[:], w2v(jp, m), h8v(jp),
                        start=(jp == 0), stop=(jp == 3), perf_mode=DR,
                    )
                # y = psum/512 + (out' + b2 + x'): one DVE op per chunk. The
                # last chunk's eviction + DMA is the kernel tail -- split it
                # in half so the store starts sooner.
                y_t = yp.tile([P, 512], BF16, name="y_t")
                if m < CH - 1:
                    nc.vector.scalar_tensor_tensor(
                        y_t[:], ps[:], 1.0 / W8S, ox[m][:], ALU.mult, ALU.add
                    )
                    nc.scalar.dma_start(out=yt_d[:, m * R : (m + 1) * R], in_=y_t[:])
                else:
                    for h in range(2):
                        sl = slice(h * 256, (h + 1) * 256)
                        nc.vector.scalar_tensor_tensor(
                            y_t[:, sl], ps[:, sl], 1.0 / W8S, ox[m][:, sl],
                            ALU.mult, ALU.add,
                        )
                        nc.scalar.dma_start(
                            out=yt_d[:, m * R + h * 256 : m * R + (h + 1) * 256],
                            in_=y_t[:, sl],
                        )

    nc.compile()
    return nc


def _get_nc(kv_bias: bool):
    key = ("nc", kv_bias)
    if key not in _CACHE:
        _CACHE[key] = _build(kv_bias)
    return _CACHE[key]


def _pack_pf(a):
    """[CH*P, F] row-major -> [P, CH*F] (partition-chunk packing)."""
    n, f = a.shape
    ch = n // P
    return np.ascontiguousarray(a.reshape(ch, P, f).transpose(1, 0, 2).reshape(P, ch * f))


def _prep_inputs(x, Wq, bq, Wk, bk, Wv, bv, W1, b1, W2, b2, kv_bias):
    bf = ml_dtypes.bfloat16
    f8 = ml_dtypes.float8_e4m3
    def _half_block(a):
        # [P, CH*C] chunk-major -> [P, (2, CH, 512)] half-blocked
        return np.ascontiguousarray(
            a.reshape(P, CH, 2, 512).transpose(0, 2, 1, 3).reshape(P, CH * C)
        )

    wq_p = _pack_pf((Wq.T * SCALE).astype(np.float32)).astype(bf)
    wk_p = _half_block(_pack_pf(np.ascontiguousarray(Wk.T))).astype(bf)
    wv_p = _half_block(_pack_pf(np.ascontiguousarray(Wv.T))).astype(bf)
    w1_p = np.clip(_pack_pf(np.ascontiguousarray(W1.T)) * W8S, -240, 240).astype(f8)
    w2_p = np.clip(_pack_pf(np.ascontiguousarray(W2.T)) * W8S, -240, 240).astype(f8)
    bqs = np.ascontiguousarray((bq * SCALE).astype(np.float32).reshape(CH, P).T)
    b1r = np.ascontiguousarray(b1.astype(np.float32).reshape(CH, P).T)
    b2r = np.ascontiguousarray(b2.astype(np.float32).reshape(CH, P).T)

    xf = x.reshape(B * S, C)
    in_maps = []
    for core in range(NCORES):
        xs = xf[core * R : (core + 1) * R]           # [R, C]
        xt = _pack_pf(np.ascontiguousarray(xs.T))    # [P, CH*R] f32
        m = {
            "xtb": xt.astype(bf),
            "wq": wq_p,
            "wk": wk_p,
            "wv": wv_p,
            "w1": w1_p,
            "w2": w2_p,
            "bqs": bqs,
            "b1r": b1r,
            "b2r": b2r,
        }
        if kv_bias:
            m["bkr"] = bk.astype(bf).reshape(1, C)
            m["bvr"] = bv.astype(bf).reshape(1, C)
        in_maps.append(m)
    return in_maps


def _unpack_out(results):
    y = np.empty((B * S, C), np.float32)
    for core in range(NCORES):
        yt = np.asarray(results[core]["yt"], dtype=np.float32)   # [P, CH*R]
        blk = yt.reshape(P, CH, R).transpose(1, 0, 2).reshape(C, R)
        y[core * R : (core + 1) * R] = blk.T
    return y.reshape(B, S, C)


def _run(inputs, trace=False, trace_cores=None):
    x = np.asarray(inputs["x"], np.float32)
    args = [np.asarray(inputs[k], np.float32) for k in
            ("Wq", "bq", "Wk", "bk", "Wv", "bv", "W1", "b1", "W2", "b2")]
    kv_bias = bool(np.any(args[3]) or np.any(args[5]))
    nc = _get_nc(kv_bias)
    in_maps = _prep_inputs(x, *args, kv_bias)
    res = run_bass_kernel_spmd(
        nc, in_maps, core_ids=list(range(NCORES)), trace=trace,
        trace_cores=trace_cores,
    )
    return _unpack_out(res.results), res


def kernel(**inputs) -> np.ndarray:
    out, _ = _run(inputs, trace=False)
    return out


def kernel_profiled(**inputs):
    """Returns (output, exec_time_ns) using neuron-profile NTFF timing."""
    out, res = _run(inputs, trace=True)
    return out, res.exec_time_ns
